# revision 1
# baseline (speedup 1.0000x reference)
"""Trainium2 Bass kernel for nn_AttnFathers.

Reference computation:
    energy      = einsum('bmfh,kh->bmfk', FO, W) + bias          # huge matmul
    attn_energy = einsum('bh,bmfh->bmf', hidden[0], energy)
    out         = softmax(attn_energy, axis=1)                   # over m

Algebraic rewrite (exact, in fp32):
    v[b]   = hidden[b] @ W          # [H]
    c[b]   = hidden[b] . bias       # scalar
    e[b,m,f] = FO[b,m,f,:].v[b] + c[b]
    out    = softmax_m(e)

This is ~1000x fewer FLOPs and turns the kernel memory-bound: each core
streams its FO shard (67 MB) once from HBM, does one fused
tensor_tensor_reduce (multiply + row-reduce) per 128-row tile on VectorE,
and a tiny softmax per batch.

Sharding: data-parallel over batch B=16 -> 2 batches per core on 8 cores.
"""

import sys
import os

for _p in ("/opt/trn_rl_repo", "/root/.axon_site/_ro/trn_rl_repo"):
    if os.path.isdir(_p) and _p not in sys.path:
        sys.path.insert(0, _p)

import numpy as np
from contextlib import ExitStack

import concourse.bass as bass
import concourse.bacc as bacc
import concourse.tile as tile
from concourse import mybir
from concourse import bass_isa
from concourse.bass_utils import run_bass_kernel_spmd

F32 = mybir.dt.float32

B, MAX_LEN, FATHER_NUM, H = 16, 256, 32, 1024
NCORES = 8
BPC = B // NCORES                 # batches per core = 2
ROWS = MAX_LEN * FATHER_NUM       # rows per batch = 8192 (r = m*32 + f)
P = 128
TPB = ROWS // P                   # 128-row tiles per batch = 64
CHUNK_S = 1                       # row-tiles per DMA (per-slice streaming)
CPB = TPB // CHUNK_S              # slice-DMAs per batch = 64
SLICE_BUFS = 20                   # in-flight 512KB slices (10 MB)
KC = H // P                       # 128-contraction chunks = 8


def build_nc() -> bass.Bass:
    nc = bacc.Bacc(trn_type="TRN2")

    # NOTE: attn_b is deliberately absent. The bias contributes
    # hidden[b].attn_b to every logit of batch b — constant across the
    # softmax axis m (and f), so it cancels exactly in the softmax.
    fo = nc.dram_tensor("fo", [BPC, MAX_LEN, FATHER_NUM, H], F32, kind="ExternalInput")
    hid = nc.dram_tensor("hid", [BPC, H], F32, kind="ExternalInput")
    w = nc.dram_tensor("w", [H, H], F32, kind="ExternalInput")
    out = nc.dram_tensor("out", [BPC, MAX_LEN, FATHER_NUM], F32, kind="ExternalOutput")

    # Constant tensors embedded in the NEFF.
    ident_np = np.eye(P, dtype=np.float32)
    # gmat[p, f] = 1 if p % 32 == f  (sums the 4 partition groups -> 32 f-rows)
    gmat_np = np.zeros((P, FATHER_NUM), dtype=np.float32)
    gmat_np[np.arange(P), np.arange(P) % FATHER_NUM] = 1.0
    # g2[f, p] = 1 if p % 32 == f    (broadcast 32 f-rows -> 128 partitions)
    g2_np = np.ascontiguousarray(gmat_np.T)
    # bsel[k, b*128 + p] = 1 if k == b (broadcast row b of a [BPC, N] tensor)
    bsel_np = np.zeros((BPC, BPC * P), dtype=np.float32)
    for b in range(BPC):
        bsel_np[b, b * P:(b + 1) * P] = 1.0
    negones_np = np.full((1, P), -1.0, dtype=np.float32)

    ident_d = nc.inline_tensor(ident_np, "identc")
    gmat_d = nc.inline_tensor(gmat_np, "gmatc")
    g2_d = nc.inline_tensor(g2_np, "g2c")
    bsel_d = nc.inline_tensor(bsel_np, "bselc")
    negones_d = nc.inline_tensor(negones_np, "negonesc")

    with tile.TileContext(nc) as tc, ExitStack() as ctx:
        consts = ctx.enter_context(tc.tile_pool(name="consts", bufs=1))
        wpool = ctx.enter_context(tc.tile_pool(name="wpool", bufs=1))
        chunks = ctx.enter_context(tc.tile_pool(name="chunks", bufs=SLICE_BUFS))
        scratchp = ctx.enter_context(tc.tile_pool(name="scratchp", bufs=1))
        epool = ctx.enter_context(tc.tile_pool(name="epool", bufs=2))
        smallp = ctx.enter_context(tc.tile_pool(name="smallp", bufs=2))
        outp = ctx.enter_context(tc.tile_pool(name="outp", bufs=2))
        psum1 = ctx.enter_context(tc.tile_pool(name="psum1", bufs=2, space="PSUM"))
        psum2 = ctx.enter_context(tc.tile_pool(name="psum2", bufs=1, space="PSUM"))

        # ---- urgent inputs first: ident + hidden, then W split across both
        # HWDGE rings so the v-chain completes while FO slices queue behind.
        ident = consts.tile([P, P], F32)
        nc.sync.dma_start(out=ident, in_=ident_d.ap())
        hid_sb = consts.tile([BPC, H], F32)
        nc.scalar.dma_start(out=hid_sb, in_=hid.ap())

        # W as [k-partition, kc, h] so rhs chunks are wt[:, kc, n0:n1].
        # KC DMAs alternating rings; the v-matmuls consume chunks as they land.
        wt = wpool.tile([P, KC, H], F32)
        w_ap = w.ap()
        for k in range(KC):
            eng = nc.sync if k % 2 == 0 else nc.scalar
            eng.dma_start(out=wt[:, k, :], in_=w_ap[k * P:(k + 1) * P, :])

        # Softmax constants (not urgent).
        gmat = consts.tile([P, FATHER_NUM], F32)
        nc.sync.dma_start(out=gmat, in_=gmat_d.ap())
        g2 = consts.tile([FATHER_NUM, P], F32)
        nc.sync.dma_start(out=g2, in_=g2_d.ap())
        bsel = consts.tile([BPC, BPC * P], F32)
        nc.scalar.dma_start(out=bsel, in_=bsel_d.ap())
        negones = consts.tile([1, P], F32)
        nc.scalar.dma_start(out=negones, in_=negones_d.ap())

        # Prefetch the exp activation table while DMAs stream.
        warm = consts.tile([1, 1], F32)
        nc.vector.memset(warm, 0.0)
        nc.scalar.activation(
            out=warm, in_=warm, func=mybir.ActivationFunctionType.Exp
        )

        # ---- hT = hidden^T chunks [128, KC, BPC] ---------------------------
        hT_ps = psum1.tile([P, KC, BPC], F32, tag="ps1")
        for k in range(KC):
            nc.tensor.transpose(
                hT_ps[:, k, :], hid_sb[:, k * P:(k + 1) * P], ident[0:BPC, 0:BPC]
            )
        hT = consts.tile([P, KC, BPC], F32)
        nc.vector.tensor_copy(out=hT, in_=hT_ps)

        # ---- v = hidden @ W  -> [BPC, H] -----------------------------------
        v_ps = psum2.tile([BPC, H], F32, tag="ps2")
        for k in range(KC):  # k outer: consume each W chunk as it lands
            for half in range(2):
                n0, n1 = half * 512, (half + 1) * 512
                nc.tensor.matmul(
                    v_ps[:, n0:n1], hT[:, k, :], wt[:, k, n0:n1],
                    start=(k == 0), stop=(k == KC - 1),
                )
        v_sb = consts.tile([BPC, H], F32)
        nc.vector.tensor_copy(out=v_sb, in_=v_ps)

        # ---- per-batch broadcast: v[b] -> [128, H] -------------------------
        vbc = []
        for b in range(BPC):
            vb_ps = psum2.tile([P, H], F32, tag="ps2")
            for half in range(2):
                n0, n1 = half * 512, (half + 1) * 512
                nc.tensor.matmul(
                    vb_ps[:, n0:n1], bsel[:, b * P:(b + 1) * P], v_sb[:, n0:n1],
                    start=True, stop=True,
                )
            vbc_b = consts.tile([P, H], F32, tag=f"vbc{b}")
            nc.vector.tensor_copy(out=vbc_b, in_=vb_ps)
            vbc.append(vbc_b)

        # ---- main loop: stream FO per 512KB slice, fused dot on DVE --------
        # row r = m*32 + f of FO[b]; slice t covers rows [t*128, (t+1)*128)
        fo_r = (
            fo.ap()
            .flatten_outer_dims()  # [BPC*ROWS, H]
            .rearrange("(b t p) h -> b t p h", b=BPC, t=TPB, p=P)
        )
        out_r = (
            out.ap()
            .rearrange("b m f -> b (m f)")
            .rearrange("b (t p) -> b t p", t=TPB, p=P)
        )

        scratch = scratchp.tile([P, H], F32)

        def emit_slice(b, t, e_t):
            ck = chunks.tile([P, H], F32, tag="ck")
            # Alternate the two HWDGE rings (SP / ACT) so consecutive
            # slice DMAs overlap their fixed per-DMA latency.
            eng = nc.sync if (b * TPB + t) % 2 == 0 else nc.scalar
            eng.dma_start(out=ck, in_=fo_r[b, t])
            # e[:, t] = sum_h ck[:, h] * v[b, h]  (fused on DVE)
            nc.vector.scalar_tensor_tensor(
                out=scratch,
                in0=ck,
                scalar=1.0,
                in1=vbc[b],
                op0=mybir.AluOpType.bypass,
                op1=mybir.AluOpType.mult,
                accum_out=e_t[:, t:t + 1],
            )

        def softmax_emit(b, e_t):
            # Softmax over m (free axis t + partition groups of 32).
            # Batch-global max K: constant shift per batch, valid for
            # softmax over m at every f.
            colmax = smallp.tile([P, 1], F32, tag="cmax")
            nc.vector.reduce_max(out=colmax, in_=e_t, axis=mybir.AxisListType.X)
            cm_ps = psum1.tile([1, P], F32, tag="ps1")
            nc.tensor.transpose(cm_ps, colmax, ident)
            gmax = smallp.tile([1, 1], F32, tag="gmax")
            nc.vector.reduce_max(out=gmax, in_=cm_ps, axis=mybir.AxisListType.X)

            negK_ps = psum1.tile([P, 1], F32, tag="ps1")
            nc.tensor.matmul(negK_ps, negones, gmax, start=True, stop=True)
            negK = smallp.tile([P, 1], F32, tag="negK")
            nc.scalar.copy(out=negK, in_=negK_ps)

            p_t = smallp.tile([P, TPB], F32, tag="pt")
            s_col = smallp.tile([P, 1], F32, tag="scol")
            nc.scalar.activation(
                out=p_t, in_=e_t,
                func=mybir.ActivationFunctionType.Exp,
                bias=negK, scale=1.0,
                accum_out=s_col,
            )

            s4_ps = psum1.tile([FATHER_NUM, 1], F32, tag="ps1")
            nc.tensor.matmul(s4_ps, gmat, s_col, start=True, stop=True)
            rinv = smallp.tile([FATHER_NUM, 1], F32, tag="rinv")
            nc.vector.reciprocal(out=rinv, in_=s4_ps)

            rb_ps = psum1.tile([P, 1], F32, tag="ps1")
            nc.tensor.matmul(rb_ps, g2, rinv, start=True, stop=True)

            nc.vector.tensor_scalar_mul(out=p_t, in0=p_t, scalar1=rb_ps)

            pT_ps = psum1.tile([TPB, P], F32, tag="pT")
            nc.tensor.transpose(pT_ps, p_t, ident)
            pT_sb = outp.tile([TPB, P], F32, tag="pTs")
            nc.scalar.copy(out=pT_sb, in_=pT_ps)
            nc.scalar.dma_start(out=out_r[b], in_=pT_sb)

        def softmax_segments(b, e_t):
            st = {}

            def seg1():  # colmax (V) + transpose (PE)
                colmax = smallp.tile([P, 1], F32, tag="cmax")
                st['colmax'] = colmax
                nc.vector.reduce_max(out=colmax, in_=e_t,
                                     axis=mybir.AxisListType.X)
                cm_ps = psum1.tile([1, P], F32, tag="ps1")
                st['cm_ps'] = cm_ps
                nc.tensor.transpose(cm_ps, colmax, ident)

            def seg2():  # global max (V), -K broadcast (PE+A), exp (A), s4 (PE)
                gmax = smallp.tile([1, 1], F32, tag="gmax")
                st['gmax'] = gmax
                nc.vector.reduce_max(out=gmax, in_=st['cm_ps'],
                                     axis=mybir.AxisListType.X)
                negK_ps = psum1.tile([P, 1], F32, tag="ps1")
                nc.tensor.matmul(negK_ps, negones, gmax, start=True, stop=True)
                negK = smallp.tile([P, 1], F32, tag="negK")
                nc.scalar.copy(out=negK, in_=negK_ps)
                p_t = smallp.tile([P, TPB], F32, tag="pt")
                s_col = smallp.tile([P, 1], F32, tag="scol")
                st['p_t'] = p_t
                nc.scalar.activation(
                    out=p_t, in_=e_t,
                    func=mybir.ActivationFunctionType.Exp,
                    bias=negK, scale=1.0,
                    accum_out=s_col,
                )
                s4_ps = psum1.tile([FATHER_NUM, 1], F32, tag="ps1")
                st['s4_ps'] = s4_ps
                nc.tensor.matmul(s4_ps, gmat, s_col, start=True, stop=True)

            def seg3():  # reciprocal (V) + broadcast (PE)
                rinv = smallp.tile([FATHER_NUM, 1], F32, tag="rinv")
                nc.vector.reciprocal(out=rinv, in_=st['s4_ps'])
                rb_ps = psum1.tile([P, 1], F32, tag="ps1")
                st['rb_ps'] = rb_ps
                nc.tensor.matmul(rb_ps, g2, rinv, start=True, stop=True)

            def seg4():  # normalize (V), transpose (PE), copy (A), store
                nc.vector.tensor_scalar_mul(out=st['p_t'], in0=st['p_t'],
                                            scalar1=st['rb_ps'])
                pT_ps = psum1.tile([TPB, P], F32, tag="pT")
                nc.tensor.transpose(pT_ps, st['p_t'], ident)
                pT_sb = outp.tile([TPB, P], F32, tag="pTs")
                nc.scalar.copy(out=pT_sb, in_=pT_ps)
                nc.scalar.dma_start(out=out_r[b], in_=pT_sb)

            return [seg1, seg2, seg3, seg4]

        # Batch 0 slices; its softmax segments hide inside batch 1's stream.
        e_t0 = epool.tile([P, TPB], F32, tag="e")
        for t in range(TPB):
            emit_slice(0, t, e_t0)
        segs0 = softmax_segments(0, e_t0)
        seg_at = {3: 0, 8: 1, 13: 2, 18: 3}
        e_t1 = epool.tile([P, TPB], F32, tag="e")
        for t in range(TPB):
            emit_slice(1, t, e_t1)
            if t in seg_at:
                segs0[seg_at[t]]()
        # Batch 1 softmax: compact tail.
        softmax_emit(1, e_t1)

    nc.compile()
    return nc


_NC_CACHE = None


def _get_nc():
    global _NC_CACHE
    if _NC_CACHE is None:
        _NC_CACHE = build_nc()
    return _NC_CACHE


def _make_in_maps(hidden, fathers_outputs, attn_W, attn_b):
    hidden = np.asarray(hidden, dtype=np.float32)
    fathers_outputs = np.asarray(fathers_outputs, dtype=np.float32)
    attn_W = np.ascontiguousarray(np.asarray(attn_W, dtype=np.float32))
    in_maps = []
    for i in range(NCORES):
        b0 = i * BPC
        in_maps.append({
            "fo": np.ascontiguousarray(fathers_outputs[b0:b0 + BPC]),
            "hid": np.ascontiguousarray(hidden[0, b0:b0 + BPC]),
            "w": attn_W,
        })
    return in_maps


def run(hidden, fathers_outputs, fathers_lengths, attn_W, attn_b, trace=False):
    """Run on the 8 NeuronCores; returns (full_output, BassKernelResults)."""
    nc = _get_nc()
    in_maps = _make_in_maps(hidden, fathers_outputs, attn_W, attn_b)
    res = run_bass_kernel_spmd(nc, in_maps, list(range(NCORES)), trace=trace)
    parts = [np.asarray(res.results[i]["out"]) for i in range(NCORES)]
    full = np.concatenate(parts, axis=0).astype(np.float32)
    return full, res


def kernel(hidden, fathers_outputs, fathers_lengths, attn_W, attn_b):
    full, _ = run(hidden, fathers_outputs, fathers_lengths, attn_W, attn_b)
    return full



# revision 2
# speedup vs baseline: 1.2015x; 1.2015x over previous
"""Trainium2 Bass kernel for nn_AttnFathers.

Reference computation:
    energy      = einsum('bmfh,kh->bmfk', FO, W) + bias          # huge matmul
    attn_energy = einsum('bh,bmfh->bmf', hidden[0], energy)
    out         = softmax(attn_energy, axis=1)                   # over m

Algebraic rewrite (exact, in fp32):
    v[b]   = hidden[b] @ W          # [H]
    c[b]   = hidden[b] . bias       # scalar, cancels in softmax
    e[b,m,f] = FO[b,m,f,:].v[b] + c[b]
    out    = softmax_m(e)

This turns the kernel memory-bound: each core streams its FO shard once
from HBM and does one fused multiply+row-reduce per 128-row tile on DVE.

fp16 edition: FO/hid/W are cast to fp16 on the host (rel err ~4e-3,
within the 2e-2 gate). This halves HBM traffic (67->33.5 MB per core)
and enables the DVE 2x perf mode (all non-scalar operands 2-byte).
Accumulation of the dot products stays f32 (accum_out), as does the
whole softmax.

Sharding: data-parallel over batch B=16 -> 2 batches per core on 8 cores.
"""

import sys
import os

for _p in ("/opt/trn_rl_repo", "/root/.axon_site/_ro/trn_rl_repo"):
    if os.path.isdir(_p) and _p not in sys.path:
        sys.path.insert(0, _p)

import numpy as np
from contextlib import ExitStack

import concourse.bass as bass
import concourse.bacc as bacc
import concourse.tile as tile
from concourse import mybir
from concourse.bass_utils import run_bass_kernel_spmd

F32 = mybir.dt.float32
F16 = mybir.dt.float16

B, MAX_LEN, FATHER_NUM, H = 16, 256, 32, 1024
NCORES = 8
BPC = B // NCORES                 # batches per core = 2
ROWS = MAX_LEN * FATHER_NUM       # rows per batch = 8192 (r = m*32 + f)
P = 128
TPB = ROWS // P                   # 128-row tiles per batch = 64
SLICE_BUFS = 28                   # in-flight 256KB fp16 slices (7 MB)
KC = H // P                       # 128-contraction chunks = 8


def build_nc() -> bass.Bass:
    nc = bacc.Bacc(trn_type="TRN2")

    # NOTE: attn_b is deliberately absent. The bias contributes
    # hidden[b].attn_b to every logit of batch b — constant across the
    # softmax axis m (and f), so it cancels exactly in the softmax.
    fo = nc.dram_tensor("fo", [BPC, MAX_LEN, FATHER_NUM, H], F16, kind="ExternalInput")
    hidT = nc.dram_tensor("hidT", [H, BPC], F16, kind="ExternalInput")
    w = nc.dram_tensor("w", [H, H], F16, kind="ExternalInput")
    out = nc.dram_tensor("out", [BPC, MAX_LEN, FATHER_NUM], F32, kind="ExternalOutput")

    # Constant tensors embedded in the NEFF.
    ident_np = np.eye(P, dtype=np.float32)
    # gmat[p, f] = 1 if p % 32 == f  (sums the 4 partition groups -> 32 f-rows)
    gmat_np = np.zeros((P, FATHER_NUM), dtype=np.float32)
    gmat_np[np.arange(P), np.arange(P) % FATHER_NUM] = 1.0
    # g2[f, p] = 1 if p % 32 == f    (broadcast 32 f-rows -> 128 partitions)
    g2_np = np.ascontiguousarray(gmat_np.T)
    # bsel[k, b*128 + p] = 1 if k == b (broadcast row b of a [BPC, N] tensor)
    bsel_np = np.zeros((BPC, BPC * P), dtype=np.float16)
    for b in range(BPC):
        bsel_np[b, b * P:(b + 1) * P] = 1.0
    negones_np = np.full((1, P), -1.0, dtype=np.float32)

    ident_d = nc.inline_tensor(ident_np, "identc")
    gmat_d = nc.inline_tensor(gmat_np, "gmatc")
    g2_d = nc.inline_tensor(g2_np, "g2c")
    bsel_d = nc.inline_tensor(bsel_np, "bselc")
    negones_d = nc.inline_tensor(negones_np, "negonesc")

    with tile.TileContext(nc) as tc, ExitStack() as ctx:
        consts = ctx.enter_context(tc.tile_pool(name="consts", bufs=1))
        wpool = ctx.enter_context(tc.tile_pool(name="wpool", bufs=1))
        chunks = ctx.enter_context(tc.tile_pool(name="chunks", bufs=SLICE_BUFS))
        scratchp = ctx.enter_context(tc.tile_pool(name="scratchp", bufs=1))
        epool = ctx.enter_context(tc.tile_pool(name="epool", bufs=2))
        smallp = ctx.enter_context(tc.tile_pool(name="smallp", bufs=2))
        outp = ctx.enter_context(tc.tile_pool(name="outp", bufs=2))
        psum1 = ctx.enter_context(tc.tile_pool(name="psum1", bufs=2, space="PSUM"))
        psum2 = ctx.enter_context(tc.tile_pool(name="psum2", bufs=1, space="PSUM"))

        # ---- urgent inputs first: hidT (tiny), then W split across both
        # HWDGE rings so the v-chain completes while FO slices queue behind.
        hT = consts.tile([P, KC, BPC], F16)
        nc.sync.dma_start(
            out=hT, in_=hidT.ap().rearrange("(k p) b -> p k b", k=KC, p=P)
        )

        # W as [k-partition, kc, h] so rhs chunks are wt[:, kc, n0:n1].
        # KC DMAs alternating rings; the v-matmuls consume chunks as they land.
        wt = wpool.tile([P, KC, H], F16)
        w_ap = w.ap()
        for k in range(KC):
            eng = nc.sync if k % 2 == 0 else nc.scalar
            eng.dma_start(out=wt[:, k, :], in_=w_ap[k * P:(k + 1) * P, :])

        # Softmax constants (not urgent).
        ident = consts.tile([P, P], F32)
        nc.sync.dma_start(out=ident, in_=ident_d.ap())
        gmat = consts.tile([P, FATHER_NUM], F32)
        nc.sync.dma_start(out=gmat, in_=gmat_d.ap())
        g2 = consts.tile([FATHER_NUM, P], F32)
        nc.sync.dma_start(out=g2, in_=g2_d.ap())
        bsel = consts.tile([BPC, BPC * P], F16)
        nc.scalar.dma_start(out=bsel, in_=bsel_d.ap())
        negones = consts.tile([1, P], F32)
        nc.scalar.dma_start(out=negones, in_=negones_d.ap())

        # Prefetch the exp activation table while DMAs stream.
        warm = consts.tile([1, 1], F32)
        nc.vector.memset(warm, 0.0)
        nc.scalar.activation(
            out=warm, in_=warm, func=mybir.ActivationFunctionType.Exp
        )

        # ---- v = hidden @ W  -> [BPC, H] -----------------------------------
        v_ps = psum2.tile([BPC, H], F32, tag="ps2")
        for k in range(KC):  # k outer: consume each W chunk as it lands
            for half in range(2):
                n0, n1 = half * 512, (half + 1) * 512
                nc.tensor.matmul(
                    v_ps[:, n0:n1], hT[:, k, :], wt[:, k, n0:n1],
                    start=(k == 0), stop=(k == KC - 1),
                )
        v_sb = consts.tile([BPC, H], F16)
        nc.vector.tensor_copy(out=v_sb, in_=v_ps)

        # ---- per-batch broadcast: v[b] -> [128, H] fp16 --------------------
        vbc = []
        for b in range(BPC):
            vb_ps = psum2.tile([P, H], F32, tag="ps2")
            for half in range(2):
                n0, n1 = half * 512, (half + 1) * 512
                nc.tensor.matmul(
                    vb_ps[:, n0:n1], bsel[:, b * P:(b + 1) * P], v_sb[:, n0:n1],
                    start=True, stop=True,
                )
            vbc_b = consts.tile([P, H], F16, tag=f"vbc{b}")
            nc.vector.tensor_copy(out=vbc_b, in_=vb_ps)
            vbc.append(vbc_b)

        # ---- main loop: stream FO per 256KB fp16 slice, fused dot on DVE ---
        # row r = m*32 + f of FO[b]; slice t covers rows [t*128, (t+1)*128)
        fo_r = (
            fo.ap()
            .flatten_outer_dims()  # [BPC*ROWS, H]
            .rearrange("(b t p) h -> b t p h", b=BPC, t=TPB, p=P)
        )
        out_r = (
            out.ap()
            .rearrange("b m f -> b (m f)")
            .rearrange("b (t p) -> b t p", t=TPB, p=P)
        )

        scratch = scratchp.tile([P, H], F16)

        def emit_slice(b, t, e_t):
            ck = chunks.tile([P, H], F16, tag="ck")
            # Alternate the two HWDGE rings (SP / ACT) so consecutive
            # slice DMAs overlap their fixed per-DMA latency.
            eng = nc.sync if (b * TPB + t) % 2 == 0 else nc.scalar
            eng.dma_start(out=ck, in_=fo_r[b, t])
            # e[:, t] = sum_h ck[:, h] * v[b, h]  (fused on DVE, 2x mode)
            nc.vector.scalar_tensor_tensor(
                out=scratch,
                in0=ck,
                scalar=1.0,
                in1=vbc[b],
                op0=mybir.AluOpType.bypass,
                op1=mybir.AluOpType.mult,
                accum_out=e_t[:, t:t + 1],
            )

        def softmax_emit(b, e_t):
            # Softmax over m (free axis t + partition groups of 32).
            # Batch-global max K: constant shift per batch, valid for
            # softmax over m at every f.
            colmax = smallp.tile([P, 1], F32, tag="cmax")
            nc.vector.reduce_max(out=colmax, in_=e_t, axis=mybir.AxisListType.X)
            cm_ps = psum1.tile([1, P], F32, tag="ps1")
            nc.tensor.transpose(cm_ps, colmax, ident)
            gmax = smallp.tile([1, 1], F32, tag="gmax")
            nc.vector.reduce_max(out=gmax, in_=cm_ps, axis=mybir.AxisListType.X)

            negK_ps = psum1.tile([P, 1], F32, tag="ps1")
            nc.tensor.matmul(negK_ps, negones, gmax, start=True, stop=True)
            negK = smallp.tile([P, 1], F32, tag="negK")
            nc.scalar.copy(out=negK, in_=negK_ps)

            p_t = smallp.tile([P, TPB], F32, tag="pt")
            s_col = smallp.tile([P, 1], F32, tag="scol")
            nc.scalar.activation(
                out=p_t, in_=e_t,
                func=mybir.ActivationFunctionType.Exp,
                bias=negK, scale=1.0,
                accum_out=s_col,
            )

            s4_ps = psum1.tile([FATHER_NUM, 1], F32, tag="ps1")
            nc.tensor.matmul(s4_ps, gmat, s_col, start=True, stop=True)
            rinv = smallp.tile([FATHER_NUM, 1], F32, tag="rinv")
            nc.vector.reciprocal(out=rinv, in_=s4_ps)

            rb_ps = psum1.tile([P, 1], F32, tag="ps1")
            nc.tensor.matmul(rb_ps, g2, rinv, start=True, stop=True)

            nc.vector.tensor_scalar_mul(out=p_t, in0=p_t, scalar1=rb_ps)

            pT_ps = psum1.tile([TPB, P], F32, tag="pT")
            nc.tensor.transpose(pT_ps, p_t, ident)
            pT_sb = outp.tile([TPB, P], F32, tag="pTs")
            nc.scalar.copy(out=pT_sb, in_=pT_ps)
            nc.scalar.dma_start(out=out_r[b], in_=pT_sb)

        def softmax_segments(b, e_t):
            st = {}

            def seg1():  # colmax (V) + transpose (PE)
                colmax = smallp.tile([P, 1], F32, tag="cmax")
                st['colmax'] = colmax
                nc.vector.reduce_max(out=colmax, in_=e_t,
                                     axis=mybir.AxisListType.X)
                cm_ps = psum1.tile([1, P], F32, tag="ps1")
                st['cm_ps'] = cm_ps
                nc.tensor.transpose(cm_ps, colmax, ident)

            def seg2():  # global max (V), -K broadcast (PE+A), exp (A), s4 (PE)
                gmax = smallp.tile([1, 1], F32, tag="gmax")
                st['gmax'] = gmax
                nc.vector.reduce_max(out=gmax, in_=st['cm_ps'],
                                     axis=mybir.AxisListType.X)
                negK_ps = psum1.tile([P, 1], F32, tag="ps1")
                nc.tensor.matmul(negK_ps, negones, gmax, start=True, stop=True)
                negK = smallp.tile([P, 1], F32, tag="negK")
                nc.scalar.copy(out=negK, in_=negK_ps)
                p_t = smallp.tile([P, TPB], F32, tag="pt")
                s_col = smallp.tile([P, 1], F32, tag="scol")
                st['p_t'] = p_t
                nc.scalar.activation(
                    out=p_t, in_=e_t,
                    func=mybir.ActivationFunctionType.Exp,
                    bias=negK, scale=1.0,
                    accum_out=s_col,
                )
                s4_ps = psum1.tile([FATHER_NUM, 1], F32, tag="ps1")
                st['s4_ps'] = s4_ps
                nc.tensor.matmul(s4_ps, gmat, s_col, start=True, stop=True)

            def seg3():  # reciprocal (V) + broadcast (PE)
                rinv = smallp.tile([FATHER_NUM, 1], F32, tag="rinv")
                nc.vector.reciprocal(out=rinv, in_=st['s4_ps'])
                rb_ps = psum1.tile([P, 1], F32, tag="ps1")
                st['rb_ps'] = rb_ps
                nc.tensor.matmul(rb_ps, g2, rinv, start=True, stop=True)

            def seg4():  # normalize (V), transpose (PE), copy (A), store
                nc.vector.tensor_scalar_mul(out=st['p_t'], in0=st['p_t'],
                                            scalar1=st['rb_ps'])
                pT_ps = psum1.tile([TPB, P], F32, tag="pT")
                nc.tensor.transpose(pT_ps, st['p_t'], ident)
                pT_sb = outp.tile([TPB, P], F32, tag="pTs")
                nc.scalar.copy(out=pT_sb, in_=pT_ps)
                nc.scalar.dma_start(out=out_r[b], in_=pT_sb)

            return [seg1, seg2, seg3, seg4]

        # Batch 0 slices; its softmax segments hide inside batch 1's stream.
        e_t0 = epool.tile([P, TPB], F32, tag="e")
        for t in range(TPB):
            emit_slice(0, t, e_t0)
        segs0 = softmax_segments(0, e_t0)
        seg_at = {3: 0, 8: 1, 13: 2, 18: 3}
        e_t1 = epool.tile([P, TPB], F32, tag="e")
        for t in range(TPB):
            emit_slice(1, t, e_t1)
            if t in seg_at:
                segs0[seg_at[t]]()
        # Batch 1 softmax: compact tail.
        softmax_emit(1, e_t1)

    nc.compile()
    return nc


_NC_CACHE = None


def _get_nc():
    global _NC_CACHE
    if _NC_CACHE is None:
        _NC_CACHE = build_nc()
    return _NC_CACHE


def _make_in_maps(hidden, fathers_outputs, attn_W, attn_b):
    hidden = np.asarray(hidden, dtype=np.float32)
    fo16 = np.asarray(fathers_outputs, dtype=np.float32).astype(np.float16)
    w16 = np.ascontiguousarray(np.asarray(attn_W, dtype=np.float32).astype(np.float16))
    in_maps = []
    for i in range(NCORES):
        b0 = i * BPC
        in_maps.append({
            "fo": np.ascontiguousarray(fo16[b0:b0 + BPC]),
            "hidT": np.ascontiguousarray(
                hidden[0, b0:b0 + BPC].T.astype(np.float16)
            ),
            "w": w16,
        })
    return in_maps


def run(hidden, fathers_outputs, fathers_lengths, attn_W, attn_b, trace=False):
    """Run on the 8 NeuronCores; returns (full_output, BassKernelResults)."""
    nc = _get_nc()
    in_maps = _make_in_maps(hidden, fathers_outputs, attn_W, attn_b)
    res = run_bass_kernel_spmd(nc, in_maps, list(range(NCORES)), trace=trace)
    parts = [np.asarray(res.results[i]["out"]) for i in range(NCORES)]
    full = np.concatenate(parts, axis=0).astype(np.float32)
    return full, res


def kernel(hidden, fathers_outputs, fathers_lengths, attn_W, attn_b):
    full, _ = run(hidden, fathers_outputs, fathers_lengths, attn_W, attn_b)
    return full
